# revision 22
# baseline (speedup 1.0000x reference)
"""KMeans assignment kernel (retrieval_knn) for 8 Trainium2 NeuronCores.

Computes argmin_k ||x_n - c_k||^2 for x [262144, 64] f32 against
centers [1024, 64] f32, returning int32 cluster ids [262144].

Strategy (data-parallel over points, centers replicated):
  argmin_k ||x-c||^2 == argmax_k s_k,  s_k = 2*x.c_k - ||c_k||^2
  (the ||x||^2 term is constant per row; all distances are strictly
  positive for this data so the reference's abs() is a no-op).

  The dot products run on the PE in bf16 hi/lo split form:
      2x.c ~= xhi.chi + xlo.chi + xhi.clo     (error ~1e-4 absolute)
  packed as two matmuls per 512-center chunk:
      MM-A: [xhi;xlo] (128-contract) x [chi;chi]
      MM-B: xhi       (64-contract)  x [clo]
  The -||c||^2 row is folded into the DVE pass:
      tensor_tensor_reduce: s = psum + (-cn) broadcast, accum = rowwise max
  then max_index recovers the (first) index of the max -> argmin id.
"""

import numpy as np
import ml_dtypes

N_POINTS = 262144
N_FEATURES = 64
N_CLUSTERS = 1024
N_CORES = 8
PTS_PER_CORE = N_POINTS // N_CORES      # 32768
TILE_P = 128                            # points per tile (partition dim)
N_TILES = PTS_PER_CORE // TILE_P        # 256
KH = 512                                # centers per matmul chunk

_CACHE = {}


def _build_bass():
    import concourse.bass as bass
    import concourse.bacc as bacc
    import concourse.mybir as mybir
    import concourse.tile as tile
    from contextlib import ExitStack

    bf16 = mybir.dt.bfloat16
    f32 = mybir.dt.float32
    u32 = mybir.dt.uint32

    nc = bacc.Bacc(None, target_bir_lowering=False)

    xpack = nc.declare_dram_parameter("xpack", [128, PTS_PER_CORE], bf16, isOutput=False)
    cc = nc.declare_dram_parameter("cc", [128, N_CLUSTERS], bf16, isOutput=False)
    cloa = nc.declare_dram_parameter("cloa", [67, N_CLUSTERS], bf16, isOutput=False)
    out = nc.declare_dram_parameter("out", [128, N_TILES], u32, isOutput=True)

    with tile.TileContext(nc) as tc, ExitStack() as ctx:
        const_pool = ctx.enter_context(tc.tile_pool(name="const", bufs=1))
        xin_pool = ctx.enter_context(tc.tile_pool(name="xin", bufs=3))
        xa_pool = ctx.enter_context(tc.tile_pool(name="xa", bufs=3))
        psum_pool = ctx.enter_context(
            tc.tile_pool(name="psum", bufs=3, space=bass.MemorySpace.PSUM)
        )
        s_pool = ctx.enter_context(tc.tile_pool(name="s", bufs=5))
        small_pool = ctx.enter_context(tc.tile_pool(name="small", bufs=6))
        out_pool = ctx.enter_context(tc.tile_pool(name="out", bufs=1))

        cc_t = const_pool.tile([128, N_CLUSTERS], bf16)
        nc.sync.dma_start(cc_t[:], cc[:])
        cloa_t = const_pool.tile([67, N_CLUSTERS], bf16)
        nc.sync.dma_start(cloa_t[:], cloa[:])

        outbuf = out_pool.tile([128, N_TILES], u32)

        XB = 4  # tiles per x load
        for tb in range(N_TILES // XB):
            xp = xin_pool.tile([128, XB, TILE_P], bf16)
            csl = slice(tb * XB * TILE_P, (tb + 1) * XB * TILE_P)
            nc.sync.dma_start(
                xp[:], xpack[:, csl].rearrange("p (b t) -> p b t", b=XB)
            )
            # second copy of the xhi rows with 3 all-ones aug rows appended
            # (stationary for the xhi.clo - cn matmul)
            xa = xa_pool.tile([67, XB, TILE_P], bf16)
            nc.sync.dma_start(
                xa[0:64], xpack[0:64, csl].rearrange("p (b t) -> p b t", b=XB)
            )
            nc.gpsimd.memset(xa[64:67], 1.0)
            for i in range(XB):
                t = tb * XB + i
                ps = psum_pool.tile([128, N_CLUSTERS], f32)
                for kh in range(N_CLUSTERS // KH):
                    ksl = slice(kh * KH, (kh + 1) * KH)
                    nc.tensor.matmul(
                        ps[:, ksl], xp[:, i, :], cc_t[:, ksl], start=True, stop=False
                    )
                    nc.tensor.matmul(
                        ps[:, ksl], xa[:, i, :], cloa_t[:, ksl],
                        start=False, stop=True,
                    )
                s_t = s_pool.tile([128, N_CLUSTERS], f32)
                for kh in range(N_CLUSTERS // KH):
                    ksl = slice(kh * KH, (kh + 1) * KH)
                    nc.scalar.copy(s_t[:, ksl], ps[:, ksl])
                m8 = small_pool.tile([128, 8], f32)
                nc.vector.max(m8[:], s_t[:])
                idx8 = small_pool.tile([128, 8], u32)
                nc.vector.max_index(idx8[:], m8[:], s_t[:])
                nc.scalar.copy(outbuf[:, t : t + 1], idx8[:, 0:1])

        nc.sync.dma_start(out[:], outbuf[:])

    nc.compile()
    return nc


def _prep(x: np.ndarray, centers: np.ndarray):
    bf16 = ml_dtypes.bfloat16
    xt = np.ascontiguousarray(x.T)                      # [64, N] f32
    xhi = xt.astype(bf16)
    xlo = (xt - xhi.astype(np.float32)).astype(bf16)
    xpack = np.concatenate([xhi, xlo], axis=0)          # [128, N] bf16

    c2t = np.ascontiguousarray((2.0 * centers).T)       # [64, K] f32
    chi = c2t.astype(bf16)
    clo = (c2t - chi.astype(np.float32)).astype(bf16)   # [64, K] bf16
    cc = np.concatenate([chi, chi], axis=0)             # [128, K] bf16

    # -||c||^2 as a 3-term bf16 cascade, matched with all-ones stationary rows
    cn = np.sum(centers.astype(np.float32) ** 2, axis=1, dtype=np.float32)
    n1 = (-cn).astype(bf16)
    r1 = -cn - n1.astype(np.float32)
    n2 = r1.astype(bf16)
    n3 = (r1 - n2.astype(np.float32)).astype(bf16)
    cloa = np.concatenate(
        [clo, n1[None, :], n2[None, :], n3[None, :]], axis=0
    )                                                   # [67, K] bf16
    return xpack, cc, cloa


def kernel(x: np.ndarray, centers: np.ndarray) -> np.ndarray:
    import sys
    if "/opt/trn_rl_repo" not in sys.path:
        sys.path.insert(0, "/opt/trn_rl_repo")
    from concourse.bass_utils import run_bass_kernel_spmd

    x = np.asarray(x, dtype=np.float32)
    centers = np.asarray(centers, dtype=np.float32)

    xpack, cc, cloa = _prep(x, centers)

    if "nc" not in _CACHE:
        _CACHE["nc"] = _build_bass()
    nc = _CACHE["nc"]

    in_maps = []
    for c in range(N_CORES):
        sl = slice(c * PTS_PER_CORE, (c + 1) * PTS_PER_CORE)
        in_maps.append(
            {
                "xpack": np.ascontiguousarray(xpack[:, sl]),
                "cc": cc,
                "cloa": cloa,
            }
        )

    res = run_bass_kernel_spmd(nc, in_maps, list(range(N_CORES)))

    outs = []
    for c in range(N_CORES):
        o = res.results[c]["out"]                       # [128, N_TILES] uint32
        outs.append(np.asarray(o).astype(np.int64).T.reshape(-1))  # point t*128+p
    ids = np.concatenate(outs)
    return ids.astype(np.int32)


if __name__ == "__main__":
    rng = np.random.default_rng(0)
    x = rng.normal(size=(N_POINTS, N_FEATURES)).astype(np.float32)
    c = rng.normal(size=(N_CLUSTERS, N_FEATURES)).astype(np.float32)
    ids = kernel(x=x, centers=c)
    d = (
        np.sum(x * x, 1)[:, None]
        - 2.0 * (x @ c.T)
        + np.sum(c * c, 1)[None, :]
    )
    ref = np.argmin(np.abs(d), axis=1)
    print("mismatch:", np.mean(ids != ref))
